# revision 35
# baseline (speedup 1.0000x reference)
"""Trainium2 Bass kernel for the FM (factorization machine) embedding-lookup model.

Computation (per batch item b):
  E[f] = emb_tables[f, feats[f,b]]          (8 fields, vocab 12, dim 64)
  S    = sum_f E[f]
  pair = 0.5*(||S||^2 - sum_f ||E[f]||^2)
  inf_k = S @ action_emb[k] + sum_f emb_first[f, feats[f,b]] + action_first[k]
  loss = mean_b sum_k pos_weights[b,k] * (inf_k - label[b,k])^2

Device strategy (8 cores, batch-sharded 16384 items/core):
  - onehot[96, n] (class c = 12f+v) via DVE is_equal of a 12x-replicated
    feats tile against a per-partition class pattern.
  - mm1 (PE, f16): T[96, 64]^T @ onehot -> S in PSUM. Chunk pairs stack
    at partition bases 0/64 so ACT squares 128 partitions at once.
  - mm2lin (PE, f16): [SQ|A0p|A1p][96, 3]^T @ onehot -> linear terms
    (-0.5*sum||E_f||^2, the full inf0/inf1) at 32-aligned per-chunk
    PSUM partitions.
  - ACT squares psum -> s2[128, n] (f16; squares of S are small and
    cancellation-free so f16 suffices).
  - mm2a (PE, f16): 0.5-vector^T @ s2 accumulates 0.5*||S||^2 onto the
    pair row (M=1, per chunk).
  - loss tail computed batch-on-partition after cheap DMA reshapes;
    per-partition partials summed on host.
"""

import numpy as np

N_FIELDS = 8
VOCAB = 12
DIM = 64
BATCH = 131072
NCORES = 8
BC = BATCH // NCORES          # 16384 items per core
CHUNK = 512                   # matmul streaming chunk (one PSUM bank)
NCHUNK = BC // CHUNK          # 32
NQUAD = NCHUNK // 4           # 8 quads of 4 chunks
NCLS = N_FIELDS * VOCAB       # 96 one-hot classes

# chunk-within-quad ci -> ps2 partition slot (32*slot), chosen so mm2lin's
# PE column group is always disjoint from mm1's (mm1 uses cols 0-63 for even
# ci, 64-127 for odd ci).
SLOT = [2, 0, 3, 1]
INV_SLOT = [1, 3, 0, 2]       # slot -> ci

_prog_cache = {}


def _bf16_hilo(w):
    import ml_dtypes

    hi = w.astype(ml_dtypes.bfloat16)
    lo = (w - hi.astype(np.float32)).astype(ml_dtypes.bfloat16)
    return hi, lo


def _build_program():
    from contextlib import ExitStack

    import concourse.tile as tile
    from concourse import bacc, mybir

    dt = mybir.dt
    op = mybir.AluOpType

    nc = bacc.Bacc("TRN2", target_bir_lowering=False, debug=False,
                   num_devices=NCORES)

    featsr = nc.dram_tensor("featsr", [NCLS, BC], dt.float16,
                            kind="ExternalInput").ap()
    vpat = nc.dram_tensor("vpat", [NCLS, 1], dt.float32,
                          kind="ExternalInput").ap()
    whi_d = nc.dram_tensor("whi", [NCLS, DIM], dt.float16,
                           kind="ExternalInput").ap()
    lhi_d = nc.dram_tensor("lhi", [NCLS, 3], dt.float16,
                           kind="ExternalInput").ap()
    half_d = nc.dram_tensor("halfw", [128, 2], dt.float16,
                            kind="ExternalInput").ap()
    label_d = nc.dram_tensor("labelw", [BC, 2], dt.float32,
                             kind="ExternalInput").ap()
    posw_d = nc.dram_tensor("posw", [BC, 2], dt.float32,
                            kind="ExternalInput").ap()

    pair_d = nc.dram_tensor("pairv", [4, NQUAD, CHUNK], dt.float32,
                            kind="ExternalOutput").ap()
    inf0_d = nc.dram_tensor("inf0", [4, NQUAD, CHUNK], dt.float32,
                            kind="ExternalOutput").ap()
    inf1_d = nc.dram_tensor("inf1", [4, NQUAD, CHUNK], dt.float32,
                            kind="ExternalOutput").ap()
    lossp_d = nc.dram_tensor("lossp", [128, 1], dt.float32,
                             kind="ExternalOutput").ap()

    with tile.TileContext(nc) as tc, ExitStack() as ctx:
        cpool = ctx.enter_context(tc.tile_pool(name="consts", bufs=1))
        fbpool = ctx.enter_context(tc.tile_pool(name="fb", bufs=1))
        ohpool = ctx.enter_context(tc.tile_pool(name="oh", bufs=3))
        s2pool = ctx.enter_context(tc.tile_pool(name="s2", bufs=3))
        rowpool = ctx.enter_context(tc.tile_pool(name="rows", bufs=1))
        losspool = ctx.enter_context(tc.tile_pool(name="loss", bufs=1))
        ps1pool = ctx.enter_context(tc.tile_pool(name="ps1", bufs=2,
                                                 space="PSUM"))
        ps2pool = ctx.enter_context(tc.tile_pool(name="ps2", bufs=2,
                                                 space="PSUM"))

        # constants
        whi = cpool.tile([NCLS, DIM], dt.float16)
        nc.sync.dma_start(whi[:], whi_d[:])
        lhi = cpool.tile([NCLS, 3], dt.float16)
        nc.sync.dma_start(lhi[:], lhi_d[:])
        halfw = cpool.tile([128, 2], dt.float16)
        nc.sync.dma_start(halfw[:], half_d[:])
        vp = cpool.tile([NCLS, 1], dt.float32)
        nc.sync.dma_start(vp[:], vpat[:])

        # replicated feats: one DMA per quad so the first compare can
        # start as soon as the first 384KB lands
        fb = fbpool.tile([NCLS, BC], dt.float16)
        FQ = BC // NQUAD
        for h in range(NQUAD):
            nc.sync.dma_start(fb[:, h * FQ:(h + 1) * FQ],
                              featsr[:, h * FQ:(h + 1) * FQ])

        rowsball = rowpool.tile([128, NQUAD * CHUNK], dt.float32)

        # PE warm-up: dense back-to-back matmuls for ~5us so the HAM
        # un-throttles the PE clock before the real stream starts. Uses whi
        # as its own rhs so it only depends on the small weight DMA.
        wups = ps1pool.tile([128, DIM], dt.float32, tag="warm")
        for i in range(18):
            nc.tensor.matmul(wups[0:64, :], whi[:], whi[:],
                             start=(i == 0), stop=(i == 17),
                             skip_group_check=True)

        for q in range(NQUAD):
            base = q * 4 * CHUNK
            oh = ohpool.tile([NCLS, 4 * CHUNK], dt.float16)
            nc.vector.tensor_scalar(oh[:], fb[:, base:base + 4 * CHUNK],
                                    vp[:], None, op.is_equal)

            ps1 = ps1pool.tile([128, 2 * CHUNK], dt.float32)
            ps2 = ps2pool.tile([128, CHUNK], dt.float32)

            def rhs(ci):
                return oh[:, ci * CHUNK:(ci + 1) * CHUNK]

            # interleave mm1/lin so consecutive matmuls target different
            # PSUM banks (same-bank pairs serialize at isolated latency;
            # cross-bank pairs pipeline at ~216ns)
            for ci in (0, 1, 2, 3):
                r0 = 64 * (ci % 2)
                c0 = CHUNK * (ci // 2)
                nc.tensor.matmul(ps1[r0:r0 + 64, c0:c0 + CHUNK], whi[:],
                                 rhs(ci), start=True, stop=True,
                                 tile_position=(0, r0))
                s32 = 32 * SLOT[ci]
                nc.tensor.matmul(ps2[s32:s32 + 3, :], lhi[:], rhs(ci),
                                 start=True, stop=False,
                                 tile_position=(0, s32),
                                 skip_group_check=True)

            s2 = s2pool.tile([128, 2 * CHUNK], dt.float16)
            nc.scalar.activation(s2[:], ps1[:],
                                 mybir.ActivationFunctionType.Square)

            for ci in (0, 2, 1, 3):
                s32 = 32 * SLOT[ci]
                c0 = CHUNK * (ci // 2)
                nc.tensor.matmul(ps2[s32:s32 + 1, :],
                                 halfw[:, ci % 2:ci % 2 + 1],
                                 s2[:, c0:c0 + CHUNK],
                                 start=False, stop=True,
                                 tile_position=(0, s32),
                                 skip_group_check=True)

            nc.vector.tensor_copy(
                rowsball[0:99, q * CHUNK:(q + 1) * CHUNK], ps2[0:99, :])

        rba = rowsball[:].rearrange("(s r) n -> s r n", r=32)

        # ---- loss-critical reshape DMAs first (parallel HWDGE rings) ----
        # partition p = 32s + 4g + h holds items of chunk 4g+INV_SLOT[s]
        inf0_bp = losspool.tile([128, 128], dt.float32)
        inf1_bp = losspool.tile([128, 128], dt.float32)
        nc.sync.dma_start(inf0_bp[:], rba[:, 1, :])
        nc.scalar.dma_start(inf1_bp[:], rba[:, 2, :])

        # ---- outputs: quantity k lives at partitions {32s+k} ----
        for k, dst in ((0, pair_d), (1, inf0_d), (2, inf1_d)):
            src = rba[:, k, :].rearrange("s (g j) -> s g j", j=CHUNK)
            nc.sync.dma_start(dst[:], src)

        lab_bp = losspool.tile([128, 256], dt.float32)
        pw_bp = losspool.tile([128, 256], dt.float32)
        for srcd, dstt in ((label_d, lab_bp), (posw_d, pw_bp)):
            srcv = srcd.rearrange("(g c y) k -> c g (y k)",
                                  g=NQUAD, c=4, y=512)
            for s in range(4):
                nc.scalar.dma_start(dstt[32 * s:32 * s + 32, :],
                                    srcv[INV_SLOT[s]])

        lab_v = lab_bp[:].rearrange("p (l k) -> p k l", k=2)
        pw_v = pw_bp[:].rearrange("p (l k) -> p k l", k=2)

        parts = []
        junk = losspool.tile([128, 128], dt.float32)
        for k, infbp in ((0, inf0_bp), (1, inf1_bp)):
            d = losspool.tile([128, 128], dt.float32, tag=f"d{k}")
            nc.vector.tensor_tensor(d[:], infbp[:], lab_v[:, k, :],
                                    op.subtract)
            e = losspool.tile([128, 128], dt.float32, tag=f"e{k}")
            nc.vector.tensor_tensor(e[:], d[:], d[:], op.mult)
            p = losspool.tile([128, 1], dt.float32, tag=f"p{k}")
            nc.vector.scalar_tensor_tensor(junk[:], e[:], 0.0, pw_v[:, k, :],
                                           op.add, op.mult, accum_out=p[:])
            parts.append(p)
        lsum = losspool.tile([128, 1], dt.float32)
        nc.vector.tensor_tensor(lsum[:], parts[0][:], parts[1][:], op.add)
        nc.sync.dma_start(lossp_d[:], lsum[:])

    nc.compile()
    return nc


def _get_program():
    if "prog" not in _prog_cache:
        _prog_cache["prog"] = _build_program()
    return _prog_cache["prog"]


def _pack_host(emb_tables, emb_first, action_emb, action_first):
    """mm1 weights [96, 64] (hi/lo bf16), mm2lin [96, 3] (hi/lo), halves."""
    T = np.asarray(emb_tables, np.float32).reshape(NCLS, DIM)
    first = np.asarray(emb_first, np.float32).reshape(NCLS)
    a0 = np.asarray(action_emb, np.float32)[0].astype(np.float64)
    a1 = np.asarray(action_emb, np.float32)[1].astype(np.float64)
    c0 = float(np.asarray(action_first, np.float32)[0, 0])
    c1 = float(np.asarray(action_first, np.float32)[1, 0])

    T64 = T.astype(np.float64)
    A0p = (T64 @ a0 + first + c0 / N_FIELDS)
    A1p = (T64 @ a1 + first + c1 / N_FIELDS)
    SQ = -0.5 * (T64 ** 2).sum(1)

    whi = T.astype(np.float16)
    lin = np.stack([SQ, A0p, A1p], axis=1).astype(np.float16)  # [96, 3]
    halfw = np.zeros((128, 2), np.float16)
    halfw[0:64, 0] = 0.5
    halfw[64:128, 1] = 0.5
    return whi, lin, halfw


def _run(inputs, trace=False):
    import ml_dtypes
    from concourse.bass_utils import run_bass_kernel_spmd

    emb_tables = np.asarray(inputs["emb_tables"], np.float32)
    emb_first = np.asarray(inputs["emb_first"], np.float32)
    action_emb = np.asarray(inputs["action_emb"], np.float32)
    action_first = np.asarray(inputs["action_first"], np.float32)
    label = np.asarray(inputs["label"], np.float32)
    pos_weights = np.asarray(inputs["pos_weights"], np.float32)
    feats = np.asarray(inputs["feats"]).astype(np.int32)

    whi, lhi, halfw = _pack_host(emb_tables, emb_first,
                                 action_emb, action_first)
    vpat = (np.arange(NCLS) % VOCAB).astype(np.float32).reshape(NCLS, 1)
    featsr = np.repeat(feats.astype(np.float16), VOCAB, axis=0)

    in_maps = []
    for c in range(NCORES):
        s = slice(c * BC, (c + 1) * BC)
        in_maps.append({
            "featsr": np.ascontiguousarray(featsr[:, s]),
            "vpat": vpat,
            "whi": whi, "lhi": lhi, "halfw": halfw,
            "labelw": np.ascontiguousarray(label[s]),
            "posw": np.ascontiguousarray(pos_weights[s]),
        })

    nc = _get_program()
    res = run_bass_kernel_spmd(nc, in_maps, core_ids=list(range(NCORES)),
                               trace=trace)

    inf0 = np.empty(BATCH, np.float32)
    inf1 = np.empty(BATCH, np.float32)
    pairv = np.empty(BATCH, np.float32)
    loss_total = 0.0
    for c in range(NCORES):
        r = res.results[c]
        s = slice(c * BC, (c + 1) * BC)
        # arr[slot, q, j]; chunk 4q + INV_SLOT[slot] -> reorder slots by
        # SLOT so axis0 becomes ci, then item = (4q+ci)*512+j
        for name, dstarr in (("inf0", inf0), ("inf1", inf1),
                             ("pairv", pairv)):
            a = r[name][SLOT]              # a[ci, q, j]
            dstarr[s] = np.transpose(a, (1, 0, 2)).reshape(BC)
        loss_total += float(r["lossp"].sum())

    inferences = np.stack([inf0, inf1], axis=-1)
    weighted_loss = np.float32(loss_total / BATCH)
    pair = pairv.reshape(BATCH, 1)
    return (inferences, weighted_loss, pair), res


def kernel(**inputs):
    (out, _res) = _run(inputs, trace=False)
    return out


def kernel_traced(**inputs):
    """Like kernel() but also returns (exec_time_ns, trace_path, res)."""
    import sys
    import types

    if "antenv.axon_hooks" not in sys.modules:
        import antenv

        mod = types.ModuleType("antenv.axon_hooks")
        state = {"hook": None}
        mod.set_axon_ntff_profile_hook = lambda h: state.update(hook=h)
        mod.get_axon_ntff_profile_hook = lambda: state["hook"]
        sys.modules["antenv.axon_hooks"] = mod
        antenv.axon_hooks = mod
        from trn_agent_boot.trn_boot import _ntff_profile_via_ctypes

        mod.set_axon_ntff_profile_hook(
            _ntff_profile_via_ctypes("/opt/axon/libaxon_pjrt.so"))

    out, _ = _run(inputs, trace=False)   # warm (compile + load NEFF)
    out2, res = _run(inputs, trace=True)
    trace_path = (res.instructions_and_trace[1]
                  if res.instructions_and_trace else None)
    return out2, res.exec_time_ns, trace_path


# revision 36
# speedup vs baseline: 1.0946x; 1.0946x over previous
"""Trainium2 Bass kernel for the FM (factorization machine) embedding-lookup model.

Computation (per batch item b):
  E[f] = emb_tables[f, feats[f,b]]          (8 fields, vocab 12, dim 64)
  S    = sum_f E[f]
  pair = 0.5*(||S||^2 - sum_f ||E[f]||^2)
  inf_k = S @ action_emb[k] + sum_f emb_first[f, feats[f,b]] + action_first[k]
  loss = mean_b sum_k pos_weights[b,k] * (inf_k - label[b,k])^2

Device strategy (8 cores, batch-sharded 16384 items/core):
  - onehot[96, n] (class c = 12f+v) via DVE is_equal of a 12x-replicated
    feats tile against a per-partition class pattern.
  - mm1 (PE, f16): T[96, 64]^T @ onehot -> S in PSUM. Chunk pairs stack
    at partition bases 0/64 so ACT squares 128 partitions at once.
  - mm2lin (PE, f16): [SQ|A0p|A1p][96, 3]^T @ onehot -> linear terms
    (-0.5*sum||E_f||^2, the full inf0/inf1) at 32-aligned per-chunk
    PSUM partitions.
  - ACT squares psum -> s2[128, n] (f16; squares of S are small and
    cancellation-free so f16 suffices).
  - mm2a (PE, f16): 0.5-vector^T @ s2 accumulates 0.5*||S||^2 onto the
    pair row (M=1, per chunk).
  - loss tail computed batch-on-partition after cheap DMA reshapes;
    per-partition partials summed on host.
"""

import numpy as np

N_FIELDS = 8
VOCAB = 12
DIM = 64
BATCH = 131072
NCORES = 8
BC = BATCH // NCORES          # 16384 items per core
CHUNK = 512                   # matmul streaming chunk (one PSUM bank)
NCHUNK = BC // CHUNK          # 32
NQUAD = NCHUNK // 4           # 8 quads of 4 chunks
NCLS = N_FIELDS * VOCAB       # 96 one-hot classes

# chunk-within-quad ci -> ps2 partition slot (32*slot), chosen so mm2lin's
# PE column group is always disjoint from mm1's (mm1 uses cols 0-63 for even
# ci, 64-127 for odd ci).
SLOT = [2, 0, 3, 1]
INV_SLOT = [1, 3, 0, 2]       # slot -> ci

_prog_cache = {}


def _bf16_hilo(w):
    import ml_dtypes

    hi = w.astype(ml_dtypes.bfloat16)
    lo = (w - hi.astype(np.float32)).astype(ml_dtypes.bfloat16)
    return hi, lo


def _build_program():
    from contextlib import ExitStack

    import concourse.tile as tile
    from concourse import bacc, mybir

    dt = mybir.dt
    op = mybir.AluOpType

    nc = bacc.Bacc("TRN2", target_bir_lowering=False, debug=False,
                   num_devices=NCORES)

    featsr = nc.dram_tensor("featsr", [NCLS, BC], dt.float16,
                            kind="ExternalInput").ap()
    vpat = nc.dram_tensor("vpat", [NCLS, 1], dt.float32,
                          kind="ExternalInput").ap()
    whi_d = nc.dram_tensor("whi", [NCLS, DIM], dt.float16,
                           kind="ExternalInput").ap()
    lhi_d = nc.dram_tensor("lhi", [NCLS, 3], dt.float16,
                           kind="ExternalInput").ap()
    half_d = nc.dram_tensor("halfw", [128, 2], dt.float16,
                            kind="ExternalInput").ap()
    label_d = nc.dram_tensor("labelw", [BC, 2], dt.float32,
                             kind="ExternalInput").ap()
    posw_d = nc.dram_tensor("posw", [BC, 2], dt.float32,
                            kind="ExternalInput").ap()

    pair_d = nc.dram_tensor("pairv", [4, NQUAD, CHUNK], dt.float32,
                            kind="ExternalOutput").ap()
    inf0_d = nc.dram_tensor("inf0", [4, NQUAD, CHUNK], dt.float32,
                            kind="ExternalOutput").ap()
    inf1_d = nc.dram_tensor("inf1", [4, NQUAD, CHUNK], dt.float32,
                            kind="ExternalOutput").ap()
    lossp_d = nc.dram_tensor("lossp", [128, 1], dt.float32,
                             kind="ExternalOutput").ap()

    with tile.TileContext(nc) as tc, ExitStack() as ctx:
        cpool = ctx.enter_context(tc.tile_pool(name="consts", bufs=1))
        fbpool = ctx.enter_context(tc.tile_pool(name="fb", bufs=1))
        ohpool = ctx.enter_context(tc.tile_pool(name="oh", bufs=3))
        s2pool = ctx.enter_context(tc.tile_pool(name="s2", bufs=3))
        rowpool = ctx.enter_context(tc.tile_pool(name="rows", bufs=1))
        losspool = ctx.enter_context(tc.tile_pool(name="loss", bufs=1))
        ps1pool = ctx.enter_context(tc.tile_pool(name="ps1", bufs=2,
                                                 space="PSUM"))
        ps2pool = ctx.enter_context(tc.tile_pool(name="ps2", bufs=2,
                                                 space="PSUM"))

        # constants
        whi = cpool.tile([NCLS, DIM], dt.float16)
        nc.sync.dma_start(whi[:], whi_d[:])
        lhi = cpool.tile([NCLS, 3], dt.float16)
        nc.sync.dma_start(lhi[:], lhi_d[:])
        halfw = cpool.tile([128, 2], dt.float16)
        nc.sync.dma_start(halfw[:], half_d[:])
        vp = cpool.tile([NCLS, 1], dt.float32)
        nc.sync.dma_start(vp[:], vpat[:])

        # replicated feats: one DMA per quad so the first compare can
        # start as soon as the first 384KB lands
        fb = fbpool.tile([NCLS, BC], dt.float16)
        FQ = BC // NQUAD
        for h in range(NQUAD):
            nc.sync.dma_start(fb[:, h * FQ:(h + 1) * FQ],
                              featsr[:, h * FQ:(h + 1) * FQ])

        rowsball = rowpool.tile([128, NQUAD * CHUNK], dt.float32)

        # PE warm-up: dense back-to-back matmuls for ~5us so the HAM
        # un-throttles the PE clock before the real stream starts. Uses whi
        # as its own rhs so it only depends on the small weight DMA.
        wups = ps1pool.tile([128, DIM], dt.float32, tag="warm")
        for i in range(18):
            nc.tensor.matmul(wups[0:64, :], whi[:], whi[:],
                             start=(i == 0), stop=(i == 17),
                             skip_group_check=True)

        for q in range(NQUAD):
            base = q * 4 * CHUNK
            oh = ohpool.tile([NCLS, 4 * CHUNK], dt.float16)
            nc.vector.tensor_scalar(oh[:], fb[:, base:base + 4 * CHUNK],
                                    vp[:], None, op.is_equal)

            ps1 = ps1pool.tile([128, 2 * CHUNK], dt.float32)
            ps2 = ps2pool.tile([128, CHUNK], dt.float32)

            def rhs(ci):
                return oh[:, ci * CHUNK:(ci + 1) * CHUNK]

            # alternate tile positions so each LDWEIGHTS targets the array
            # half not currently draining the previous matmul
            for ci in (0, 1, 2, 3):
                r0 = 64 * (ci % 2)
                c0 = CHUNK * (ci // 2)
                nc.tensor.matmul(ps1[r0:r0 + 64, c0:c0 + CHUNK], whi[:],
                                 rhs(ci), start=True, stop=True,
                                 tile_position=(0, r0))
            for ci in range(4):
                s32 = 32 * SLOT[ci]
                nc.tensor.matmul(ps2[s32:s32 + 3, :], lhi[:], rhs(ci),
                                 start=True, stop=False,
                                 tile_position=(0, s32),
                                 skip_group_check=True)

            s2 = s2pool.tile([128, 2 * CHUNK], dt.float16)
            nc.scalar.activation(s2[:], ps1[:],
                                 mybir.ActivationFunctionType.Square)

            for ci in (0, 2, 1, 3):
                s32 = 32 * SLOT[ci]
                c0 = CHUNK * (ci // 2)
                nc.tensor.matmul(ps2[s32:s32 + 1, :],
                                 halfw[:, ci % 2:ci % 2 + 1],
                                 s2[:, c0:c0 + CHUNK],
                                 start=False, stop=True,
                                 tile_position=(0, s32),
                                 skip_group_check=True)

            nc.vector.tensor_copy(
                rowsball[0:99, q * CHUNK:(q + 1) * CHUNK], ps2[0:99, :])

        rba = rowsball[:].rearrange("(s r) n -> s r n", r=32)

        # ---- loss-critical reshape DMAs first (parallel HWDGE rings) ----
        # partition p = 32s + 4g + h holds items of chunk 4g+INV_SLOT[s]
        inf0_bp = losspool.tile([128, 128], dt.float32)
        inf1_bp = losspool.tile([128, 128], dt.float32)
        nc.sync.dma_start(inf0_bp[:], rba[:, 1, :])
        nc.scalar.dma_start(inf1_bp[:], rba[:, 2, :])

        # ---- outputs: quantity k lives at partitions {32s+k} ----
        for k, dst in ((0, pair_d), (1, inf0_d), (2, inf1_d)):
            src = rba[:, k, :].rearrange("s (g j) -> s g j", j=CHUNK)
            nc.sync.dma_start(dst[:], src)

        lab_bp = losspool.tile([128, 256], dt.float32)
        pw_bp = losspool.tile([128, 256], dt.float32)
        for srcd, dstt in ((label_d, lab_bp), (posw_d, pw_bp)):
            srcv = srcd.rearrange("(g c y) k -> c g (y k)",
                                  g=NQUAD, c=4, y=512)
            for s in range(4):
                nc.scalar.dma_start(dstt[32 * s:32 * s + 32, :],
                                    srcv[INV_SLOT[s]])

        lab_v = lab_bp[:].rearrange("p (l k) -> p k l", k=2)
        pw_v = pw_bp[:].rearrange("p (l k) -> p k l", k=2)

        parts = []
        junk = losspool.tile([128, 128], dt.float32)
        for k, infbp in ((0, inf0_bp), (1, inf1_bp)):
            d = losspool.tile([128, 128], dt.float32, tag=f"d{k}")
            nc.vector.tensor_tensor(d[:], infbp[:], lab_v[:, k, :],
                                    op.subtract)
            e = losspool.tile([128, 128], dt.float32, tag=f"e{k}")
            nc.vector.tensor_tensor(e[:], d[:], d[:], op.mult)
            p = losspool.tile([128, 1], dt.float32, tag=f"p{k}")
            nc.vector.scalar_tensor_tensor(junk[:], e[:], 0.0, pw_v[:, k, :],
                                           op.add, op.mult, accum_out=p[:])
            parts.append(p)
        lsum = losspool.tile([128, 1], dt.float32)
        nc.vector.tensor_tensor(lsum[:], parts[0][:], parts[1][:], op.add)
        nc.sync.dma_start(lossp_d[:], lsum[:])

    nc.compile()
    return nc


def _get_program():
    if "prog" not in _prog_cache:
        _prog_cache["prog"] = _build_program()
    return _prog_cache["prog"]


def _pack_host(emb_tables, emb_first, action_emb, action_first):
    """mm1 weights [96, 64] (hi/lo bf16), mm2lin [96, 3] (hi/lo), halves."""
    T = np.asarray(emb_tables, np.float32).reshape(NCLS, DIM)
    first = np.asarray(emb_first, np.float32).reshape(NCLS)
    a0 = np.asarray(action_emb, np.float32)[0].astype(np.float64)
    a1 = np.asarray(action_emb, np.float32)[1].astype(np.float64)
    c0 = float(np.asarray(action_first, np.float32)[0, 0])
    c1 = float(np.asarray(action_first, np.float32)[1, 0])

    T64 = T.astype(np.float64)
    A0p = (T64 @ a0 + first + c0 / N_FIELDS)
    A1p = (T64 @ a1 + first + c1 / N_FIELDS)
    SQ = -0.5 * (T64 ** 2).sum(1)

    whi = T.astype(np.float16)
    lin = np.stack([SQ, A0p, A1p], axis=1).astype(np.float16)  # [96, 3]
    halfw = np.zeros((128, 2), np.float16)
    halfw[0:64, 0] = 0.5
    halfw[64:128, 1] = 0.5
    return whi, lin, halfw


def _run(inputs, trace=False):
    import ml_dtypes
    from concourse.bass_utils import run_bass_kernel_spmd

    emb_tables = np.asarray(inputs["emb_tables"], np.float32)
    emb_first = np.asarray(inputs["emb_first"], np.float32)
    action_emb = np.asarray(inputs["action_emb"], np.float32)
    action_first = np.asarray(inputs["action_first"], np.float32)
    label = np.asarray(inputs["label"], np.float32)
    pos_weights = np.asarray(inputs["pos_weights"], np.float32)
    feats = np.asarray(inputs["feats"]).astype(np.int32)

    whi, lhi, halfw = _pack_host(emb_tables, emb_first,
                                 action_emb, action_first)
    vpat = (np.arange(NCLS) % VOCAB).astype(np.float32).reshape(NCLS, 1)
    featsr = np.repeat(feats.astype(np.float16), VOCAB, axis=0)

    in_maps = []
    for c in range(NCORES):
        s = slice(c * BC, (c + 1) * BC)
        in_maps.append({
            "featsr": np.ascontiguousarray(featsr[:, s]),
            "vpat": vpat,
            "whi": whi, "lhi": lhi, "halfw": halfw,
            "labelw": np.ascontiguousarray(label[s]),
            "posw": np.ascontiguousarray(pos_weights[s]),
        })

    nc = _get_program()
    res = run_bass_kernel_spmd(nc, in_maps, core_ids=list(range(NCORES)),
                               trace=trace)

    inf0 = np.empty(BATCH, np.float32)
    inf1 = np.empty(BATCH, np.float32)
    pairv = np.empty(BATCH, np.float32)
    loss_total = 0.0
    for c in range(NCORES):
        r = res.results[c]
        s = slice(c * BC, (c + 1) * BC)
        # arr[slot, q, j]; chunk 4q + INV_SLOT[slot] -> reorder slots by
        # SLOT so axis0 becomes ci, then item = (4q+ci)*512+j
        for name, dstarr in (("inf0", inf0), ("inf1", inf1),
                             ("pairv", pairv)):
            a = r[name][SLOT]              # a[ci, q, j]
            dstarr[s] = np.transpose(a, (1, 0, 2)).reshape(BC)
        loss_total += float(r["lossp"].sum())

    inferences = np.stack([inf0, inf1], axis=-1)
    weighted_loss = np.float32(loss_total / BATCH)
    pair = pairv.reshape(BATCH, 1)
    return (inferences, weighted_loss, pair), res


def kernel(**inputs):
    (out, _res) = _run(inputs, trace=False)
    return out


def kernel_traced(**inputs):
    """Like kernel() but also returns (exec_time_ns, trace_path, res)."""
    import sys
    import types

    if "antenv.axon_hooks" not in sys.modules:
        import antenv

        mod = types.ModuleType("antenv.axon_hooks")
        state = {"hook": None}
        mod.set_axon_ntff_profile_hook = lambda h: state.update(hook=h)
        mod.get_axon_ntff_profile_hook = lambda: state["hook"]
        sys.modules["antenv.axon_hooks"] = mod
        antenv.axon_hooks = mod
        from trn_agent_boot.trn_boot import _ntff_profile_via_ctypes

        mod.set_axon_ntff_profile_hook(
            _ntff_profile_via_ctypes("/opt/axon/libaxon_pjrt.so"))

    out, _ = _run(inputs, trace=False)   # warm (compile + load NEFF)
    out2, res = _run(inputs, trace=True)
    trace_path = (res.instructions_and_trace[1]
                  if res.instructions_and_trace else None)
    return out2, res.exec_time_ns, trace_path


# revision 37
# speedup vs baseline: 1.1098x; 1.0139x over previous
"""Trainium2 Bass kernel for the FM (factorization machine) embedding-lookup model.

Computation (per batch item b):
  E[f] = emb_tables[f, feats[f,b]]          (8 fields, vocab 12, dim 64)
  S    = sum_f E[f]
  pair = 0.5*(||S||^2 - sum_f ||E[f]||^2)
  inf_k = S @ action_emb[k] + sum_f emb_first[f, feats[f,b]] + action_first[k]
  loss = mean_b sum_k pos_weights[b,k] * (inf_k - label[b,k])^2

Device strategy (8 cores, batch-sharded 16384 items/core):
  - onehot[96, n] (class c = 12f+v) via DVE is_equal of a 12x-replicated
    feats tile against a per-partition class pattern.
  - mm1 (PE, f16): T[96, 64]^T @ onehot -> S in PSUM. Chunk pairs stack
    at partition bases 0/64 so ACT squares 128 partitions at once.
  - mm2lin (PE, f16): [SQ|A0p|A1p][96, 3]^T @ onehot -> linear terms
    (-0.5*sum||E_f||^2, the full inf0/inf1) at 32-aligned per-chunk
    PSUM partitions.
  - ACT squares psum -> s2[128, n] (f16; squares of S are small and
    cancellation-free so f16 suffices).
  - mm2a (PE, f16): 0.5-vector^T @ s2 accumulates 0.5*||S||^2 onto the
    pair row (M=1, per chunk).
  - loss tail computed batch-on-partition after cheap DMA reshapes;
    per-partition partials summed on host.
"""

import numpy as np

N_FIELDS = 8
VOCAB = 12
DIM = 64
BATCH = 131072
NCORES = 8
BC = BATCH // NCORES          # 16384 items per core
CHUNK = 512                   # matmul streaming chunk (one PSUM bank)
NCHUNK = BC // CHUNK          # 32
NQUAD = NCHUNK // 4           # 8 quads of 4 chunks
NCLS = N_FIELDS * VOCAB       # 96 one-hot classes

# chunk-within-quad ci -> ps2 partition slot (32*slot), chosen so mm2lin's
# PE column group is always disjoint from mm1's (mm1 uses cols 0-63 for even
# ci, 64-127 for odd ci).
SLOT = [2, 0, 3, 1]
INV_SLOT = [1, 3, 0, 2]       # slot -> ci

_prog_cache = {}


def _bf16_hilo(w):
    import ml_dtypes

    hi = w.astype(ml_dtypes.bfloat16)
    lo = (w - hi.astype(np.float32)).astype(ml_dtypes.bfloat16)
    return hi, lo


def _build_program():
    from contextlib import ExitStack

    import concourse.tile as tile
    from concourse import bacc, mybir

    dt = mybir.dt
    op = mybir.AluOpType

    nc = bacc.Bacc("TRN2", target_bir_lowering=False, debug=False,
                   num_devices=NCORES)

    featsr = nc.dram_tensor("featsr", [NCLS, BC], dt.float16,
                            kind="ExternalInput").ap()
    vpat = nc.dram_tensor("vpat", [NCLS, 1], dt.float32,
                          kind="ExternalInput").ap()
    whi_d = nc.dram_tensor("whi", [NCLS, DIM], dt.float16,
                           kind="ExternalInput").ap()
    lhi_d = nc.dram_tensor("lhi", [NCLS, 3], dt.float16,
                           kind="ExternalInput").ap()
    half_d = nc.dram_tensor("halfw", [128, 2], dt.float16,
                            kind="ExternalInput").ap()
    label_d = nc.dram_tensor("labelw", [BC, 2], dt.float32,
                             kind="ExternalInput").ap()
    posw_d = nc.dram_tensor("posw", [BC, 2], dt.float32,
                            kind="ExternalInput").ap()

    pair_d = nc.dram_tensor("pairv", [4, NQUAD, CHUNK], dt.float32,
                            kind="ExternalOutput").ap()
    inf0_d = nc.dram_tensor("inf0", [4, NQUAD, CHUNK], dt.float32,
                            kind="ExternalOutput").ap()
    inf1_d = nc.dram_tensor("inf1", [4, NQUAD, CHUNK], dt.float32,
                            kind="ExternalOutput").ap()
    lossp_d = nc.dram_tensor("lossp", [128, 1], dt.float32,
                             kind="ExternalOutput").ap()

    with tile.TileContext(nc) as tc, ExitStack() as ctx:
        cpool = ctx.enter_context(tc.tile_pool(name="consts", bufs=1))
        fbpool = ctx.enter_context(tc.tile_pool(name="fb", bufs=1))
        ohpool = ctx.enter_context(tc.tile_pool(name="oh", bufs=3))
        s2pool = ctx.enter_context(tc.tile_pool(name="s2", bufs=3))
        rowpool = ctx.enter_context(tc.tile_pool(name="rows", bufs=1))
        losspool = ctx.enter_context(tc.tile_pool(name="loss", bufs=1))
        ps1pool = ctx.enter_context(tc.tile_pool(name="ps1", bufs=2,
                                                 space="PSUM"))
        ps2pool = ctx.enter_context(tc.tile_pool(name="ps2", bufs=2,
                                                 space="PSUM"))

        # constants: whi first on the sync ring (warm-up dependency);
        # the rest go on the scalar ring so the sync ring is purely the
        # compute-critical path (whi, then feats quads)
        whi = cpool.tile([NCLS, DIM], dt.float16)
        nc.sync.dma_start(whi[:], whi_d[:])
        lhi = cpool.tile([NCLS, 3], dt.float16)
        nc.scalar.dma_start(lhi[:], lhi_d[:])
        halfw = cpool.tile([128, 2], dt.float16)
        nc.scalar.dma_start(halfw[:], half_d[:])
        vp = cpool.tile([NCLS, 1], dt.float32)
        nc.scalar.dma_start(vp[:], vpat[:])

        # replicated feats: one DMA per quad so the first compare can
        # start as soon as the first 384KB lands
        fb = fbpool.tile([NCLS, BC], dt.float16)
        FQ = BC // NQUAD
        for h in range(NQUAD):
            nc.sync.dma_start(fb[:, h * FQ:(h + 1) * FQ],
                              featsr[:, h * FQ:(h + 1) * FQ])

        rowsball = rowpool.tile([128, NQUAD * CHUNK], dt.float32)

        # PE warm-up: dense back-to-back matmuls for ~5us so the HAM
        # un-throttles the PE clock before the real stream starts. Uses whi
        # as its own rhs so it only depends on the small weight DMA.
        wups = ps1pool.tile([128, DIM], dt.float32, tag="warm")
        for i in range(18):
            nc.tensor.matmul(wups[0:64, :], whi[:], whi[:],
                             start=(i == 0), stop=(i == 17),
                             skip_group_check=True)

        for q in range(NQUAD):
            base = q * 4 * CHUNK
            oh = ohpool.tile([NCLS, 4 * CHUNK], dt.float16)
            nc.vector.tensor_scalar(oh[:], fb[:, base:base + 4 * CHUNK],
                                    vp[:], None, op.is_equal)

            ps1 = ps1pool.tile([128, 2 * CHUNK], dt.float32)
            ps2 = ps2pool.tile([128, CHUNK], dt.float32)

            def rhs(ci):
                return oh[:, ci * CHUNK:(ci + 1) * CHUNK]

            # alternate tile positions so each LDWEIGHTS targets the array
            # half not currently draining the previous matmul
            for ci in (0, 1, 2, 3):
                r0 = 64 * (ci % 2)
                c0 = CHUNK * (ci // 2)
                nc.tensor.matmul(ps1[r0:r0 + 64, c0:c0 + CHUNK], whi[:],
                                 rhs(ci), start=True, stop=True,
                                 tile_position=(0, r0))
            for ci in range(4):
                s32 = 32 * SLOT[ci]
                nc.tensor.matmul(ps2[s32:s32 + 3, :], lhi[:], rhs(ci),
                                 start=True, stop=False,
                                 tile_position=(0, s32),
                                 skip_group_check=True)

            s2 = s2pool.tile([128, 2 * CHUNK], dt.float16)
            nc.scalar.activation(s2[:], ps1[:],
                                 mybir.ActivationFunctionType.Square)

            for ci in (0, 2, 1, 3):
                s32 = 32 * SLOT[ci]
                c0 = CHUNK * (ci // 2)
                nc.tensor.matmul(ps2[s32:s32 + 1, :],
                                 halfw[:, ci % 2:ci % 2 + 1],
                                 s2[:, c0:c0 + CHUNK],
                                 start=False, stop=True,
                                 tile_position=(0, s32),
                                 skip_group_check=True)

            nc.vector.tensor_copy(
                rowsball[0:99, q * CHUNK:(q + 1) * CHUNK], ps2[0:99, :])

        rba = rowsball[:].rearrange("(s r) n -> s r n", r=32)

        # ---- loss-critical reshape DMAs first (parallel HWDGE rings) ----
        # partition p = 32s + 4g + h holds items of chunk 4g+INV_SLOT[s]
        inf0_bp = losspool.tile([128, 128], dt.float32)
        inf1_bp = losspool.tile([128, 128], dt.float32)
        nc.sync.dma_start(inf0_bp[:], rba[:, 1, :])
        nc.scalar.dma_start(inf1_bp[:], rba[:, 2, :])

        # ---- outputs: quantity k lives at partitions {32s+k} ----
        for k, dst in ((0, pair_d), (1, inf0_d), (2, inf1_d)):
            src = rba[:, k, :].rearrange("s (g j) -> s g j", j=CHUNK)
            nc.sync.dma_start(dst[:], src)

        lab_bp = losspool.tile([128, 256], dt.float32)
        pw_bp = losspool.tile([128, 256], dt.float32)
        for srcd, dstt in ((label_d, lab_bp), (posw_d, pw_bp)):
            srcv = srcd.rearrange("(g c y) k -> c g (y k)",
                                  g=NQUAD, c=4, y=512)
            for s in range(4):
                nc.scalar.dma_start(dstt[32 * s:32 * s + 32, :],
                                    srcv[INV_SLOT[s]])

        lab_v = lab_bp[:].rearrange("p (l k) -> p k l", k=2)
        pw_v = pw_bp[:].rearrange("p (l k) -> p k l", k=2)

        parts = []
        junk = losspool.tile([128, 128], dt.float32)
        for k, infbp in ((0, inf0_bp), (1, inf1_bp)):
            d = losspool.tile([128, 128], dt.float32, tag=f"d{k}")
            nc.vector.tensor_tensor(d[:], infbp[:], lab_v[:, k, :],
                                    op.subtract)
            e = losspool.tile([128, 128], dt.float32, tag=f"e{k}")
            nc.vector.tensor_tensor(e[:], d[:], d[:], op.mult)
            p = losspool.tile([128, 1], dt.float32, tag=f"p{k}")
            nc.vector.scalar_tensor_tensor(junk[:], e[:], 0.0, pw_v[:, k, :],
                                           op.add, op.mult, accum_out=p[:])
            parts.append(p)
        lsum = losspool.tile([128, 1], dt.float32)
        nc.vector.tensor_tensor(lsum[:], parts[0][:], parts[1][:], op.add)
        nc.sync.dma_start(lossp_d[:], lsum[:])

    nc.compile()
    return nc


def _get_program():
    if "prog" not in _prog_cache:
        _prog_cache["prog"] = _build_program()
    return _prog_cache["prog"]


def _pack_host(emb_tables, emb_first, action_emb, action_first):
    """mm1 weights [96, 64] (hi/lo bf16), mm2lin [96, 3] (hi/lo), halves."""
    T = np.asarray(emb_tables, np.float32).reshape(NCLS, DIM)
    first = np.asarray(emb_first, np.float32).reshape(NCLS)
    a0 = np.asarray(action_emb, np.float32)[0].astype(np.float64)
    a1 = np.asarray(action_emb, np.float32)[1].astype(np.float64)
    c0 = float(np.asarray(action_first, np.float32)[0, 0])
    c1 = float(np.asarray(action_first, np.float32)[1, 0])

    T64 = T.astype(np.float64)
    A0p = (T64 @ a0 + first + c0 / N_FIELDS)
    A1p = (T64 @ a1 + first + c1 / N_FIELDS)
    SQ = -0.5 * (T64 ** 2).sum(1)

    whi = T.astype(np.float16)
    lin = np.stack([SQ, A0p, A1p], axis=1).astype(np.float16)  # [96, 3]
    halfw = np.zeros((128, 2), np.float16)
    halfw[0:64, 0] = 0.5
    halfw[64:128, 1] = 0.5
    return whi, lin, halfw


def _run(inputs, trace=False):
    import ml_dtypes
    from concourse.bass_utils import run_bass_kernel_spmd

    emb_tables = np.asarray(inputs["emb_tables"], np.float32)
    emb_first = np.asarray(inputs["emb_first"], np.float32)
    action_emb = np.asarray(inputs["action_emb"], np.float32)
    action_first = np.asarray(inputs["action_first"], np.float32)
    label = np.asarray(inputs["label"], np.float32)
    pos_weights = np.asarray(inputs["pos_weights"], np.float32)
    feats = np.asarray(inputs["feats"]).astype(np.int32)

    whi, lhi, halfw = _pack_host(emb_tables, emb_first,
                                 action_emb, action_first)
    vpat = (np.arange(NCLS) % VOCAB).astype(np.float32).reshape(NCLS, 1)
    featsr = np.repeat(feats.astype(np.float16), VOCAB, axis=0)

    in_maps = []
    for c in range(NCORES):
        s = slice(c * BC, (c + 1) * BC)
        in_maps.append({
            "featsr": np.ascontiguousarray(featsr[:, s]),
            "vpat": vpat,
            "whi": whi, "lhi": lhi, "halfw": halfw,
            "labelw": np.ascontiguousarray(label[s]),
            "posw": np.ascontiguousarray(pos_weights[s]),
        })

    nc = _get_program()
    res = run_bass_kernel_spmd(nc, in_maps, core_ids=list(range(NCORES)),
                               trace=trace)

    inf0 = np.empty(BATCH, np.float32)
    inf1 = np.empty(BATCH, np.float32)
    pairv = np.empty(BATCH, np.float32)
    loss_total = 0.0
    for c in range(NCORES):
        r = res.results[c]
        s = slice(c * BC, (c + 1) * BC)
        # arr[slot, q, j]; chunk 4q + INV_SLOT[slot] -> reorder slots by
        # SLOT so axis0 becomes ci, then item = (4q+ci)*512+j
        for name, dstarr in (("inf0", inf0), ("inf1", inf1),
                             ("pairv", pairv)):
            a = r[name][SLOT]              # a[ci, q, j]
            dstarr[s] = np.transpose(a, (1, 0, 2)).reshape(BC)
        loss_total += float(r["lossp"].sum())

    inferences = np.stack([inf0, inf1], axis=-1)
    weighted_loss = np.float32(loss_total / BATCH)
    pair = pairv.reshape(BATCH, 1)
    return (inferences, weighted_loss, pair), res


def kernel(**inputs):
    (out, _res) = _run(inputs, trace=False)
    return out


def kernel_traced(**inputs):
    """Like kernel() but also returns (exec_time_ns, trace_path, res)."""
    import sys
    import types

    if "antenv.axon_hooks" not in sys.modules:
        import antenv

        mod = types.ModuleType("antenv.axon_hooks")
        state = {"hook": None}
        mod.set_axon_ntff_profile_hook = lambda h: state.update(hook=h)
        mod.get_axon_ntff_profile_hook = lambda: state["hook"]
        sys.modules["antenv.axon_hooks"] = mod
        antenv.axon_hooks = mod
        from trn_agent_boot.trn_boot import _ntff_profile_via_ctypes

        mod.set_axon_ntff_profile_hook(
            _ntff_profile_via_ctypes("/opt/axon/libaxon_pjrt.so"))

    out, _ = _run(inputs, trace=False)   # warm (compile + load NEFF)
    out2, res = _run(inputs, trace=True)
    trace_path = (res.instructions_and_trace[1]
                  if res.instructions_and_trace else None)
    return out2, res.exec_time_ns, trace_path
